# revision 21
# baseline (speedup 1.0000x reference)
"""2-layer GAT (PyG GATConv semantics) on 8 Trainium2 NeuronCores via Bass/Tile.

Contract: kernel(**inputs) takes the FULL inputs of reference.setup_inputs()
and returns the FULL [16, 4096, 128] float32 output.

v2 strategy (dst-node sharding, one SPMD program), redesigned from the v1
baseline around the measured bottlenecks (Pool/SWDGE fixed cost per indirect
DMA, DVE fixed cost per small op, f32 table bytes):

- Tables are bf16. t1 row (80 cols, 160B): [h1 (c-major: col c*H+h) | asrc1
  (H) | adst1 (H)]. t2 row (132 cols): [h2' | 1.0 | asrc2'(-c2 folded) |
  adst2' | pad]. The constant 1.0 column turns the softmax denominator into
  ordinary matmul columns.
- Per GB-block group, ONE batched indirect DMA gathers all GB*K*128 value
  rows ([h|asrc] cols only) and ONE more gathers per-edge a_dst[dst] (the
  dst rows' a_dst cols via element_offset). This cuts SWDGE generation from
  one ~1us Pool occupancy per 128 edges to one per GB*K*128 edges.
- Per-edge math is batched across the whole block on DVE in a handful of
  16-bit 2x-rate ops: the one-hot routing matrix M[e, d*K+j] (K-innermost
  layout against a host-built iota_dK constant so every operand is packed
  2-byte), logits lg = asrc[src]+adst[dst], lrelu, and the w-fold
  (L1: vf = h*w per head with c-major layout so the head-broadcast is
  packed; L2: Mw = M*w).
- Aggregation per 128-dst block: PSUM acc[d, :] += M_j^T @ rhs_j with
  rhs_j = [v*w | w] (L1) / Mw_j^T @ [h2' | 1] (L2). Softmax max-subtraction
  is skipped (logits are O(1); exact in f32 accumulation).
- ELU's -1 is folded out: t2 stores h2' = (elu+1)@W2; since sum(alpha)=1 the
  colsum(W2) correction folds into the output bias and the L2 logit constant.
- t2 shards AllGather in NSLICE slices overlapped with phase-B compute.
"""

import os
import sys

import numpy as np

if "/opt/trn_rl_repo" not in sys.path:
    sys.path.insert(0, "/opt/trn_rl_repo")

import concourse.bass as bass
import concourse.bacc as bacc
import concourse.mybir as mybir
import concourse.tile as tile

F32 = mybir.dt.float32
BF16 = mybir.dt.bfloat16
I32 = mybir.dt.int32
AOP = mybir.AluOpType
ACT = mybir.ActivationFunctionType

NEG_SLOPE = 0.2
NCORES = 8
BLK = 128

T1W = 80    # t1 row cols: [0:64] h1 (c-major), [64:72] asrc1, [72:80] adst1
T1V = 72    # value-gather cols of t1
T2W = 132   # t2 row cols: [0:128] h2', [128] one, [129] asrc2', [130] adst2', [131] pad
T2V = 130   # value-gather cols of t2
GB = 4      # blocks per batched gather instruction


class Cfg:
    def __init__(self, n_nodes, d_in, h1, c1, d2, k, nslice):
        self.N = n_nodes
        self.D = d_in
        self.H1 = h1
        self.C1 = c1
        self.D1 = h1 * c1
        self.D2 = d2
        self.K = k
        self.NSH = n_nodes // NCORES
        self.NBLK = self.NSH // BLK
        self.NSLICE = nslice
        assert self.NSH % BLK == 0 and self.NBLK % nslice == 0
        assert self.NBLK % GB == 0 and (self.NBLK // nslice) % GB == 0


def _ap(t_ap, off, dims):
    """Raw AP view of a tile slice: partition dim kept, free dims replaced.
    `off` in elements, `dims` = [[stride, size], ...]."""
    a = [list(p) for p in t_ap.ap]
    return bass.AP(t_ap.tensor, t_ap.offset + off, [a[0]] + dims)


# ---------------------------------------------------------------------------
# host-side edge schedule
# ---------------------------------------------------------------------------
def _edge_schedule(src, dst, n_nodes):
    """src/dst int64 arrays (random edges only; self-loops get a dedicated
    chunk 0 served by direct DMA). Slot (p, j) for j>=1 = random edge; the
    returned arrays are [NCORES, nblk, 128, K] with chunk 0 = self-loops."""
    nsh = n_nodes // NCORES
    nblk = nsh // BLK
    order = np.argsort(dst, kind="stable")
    src = src[order]
    dst = dst[order]
    blk_of = dst // BLK
    nblk_g = n_nodes // BLK
    counts = np.bincount(blk_of, minlength=nblk_g)
    kr = int((int(counts.max()) + 127) // 128)
    k = kr + 1
    starts = np.zeros(nblk_g + 1, dtype=np.int64)
    np.cumsum(counts, out=starts[1:])

    slots = 128 * kr
    esrc = np.zeros((NCORES, nblk, 128, k), dtype=np.int64)
    edloc = np.full((NCORES, nblk, 128, k), -1.0, dtype=np.float32)
    # chunk 0: self-loops (value/adst via direct DMA; edloc drives M)
    node = np.arange(n_nodes, dtype=np.int64).reshape(NCORES, nblk, 128)
    esrc[:, :, :, 0] = node
    edloc[:, :, :, 0] = np.tile(np.arange(128, dtype=np.float32), (NCORES, nblk, 1))
    for g in range(nblk_g):
        c, b = divmod(g, nblk)
        s0, s1 = int(starts[g]), int(starts[g + 1])
        n = s1 - s0
        flat_src = np.zeros(slots, dtype=np.int64)
        flat_loc = np.full(slots, -1.0, dtype=np.float32)
        flat_src[:n] = src[s0:s1]
        flat_loc[:n] = (dst[s0:s1] - g * BLK).astype(np.float32)
        esrc[c, b, :, 1:] = flat_src.reshape(128, kr)
        edloc[c, b, :, 1:] = flat_loc.reshape(128, kr)
    return k, esrc, edloc


def _t2_phys(cfg):
    """node id -> physical t2 row (slice-major: slice, rank, local)."""
    N, NSH, NSLICE = cfg.N, cfg.NSH, cfg.NSLICE
    sl = NSH // NSLICE
    node = np.arange(N, dtype=np.int64)
    r = node // NSH
    loc = node % NSH
    s = loc // sl
    return (s * (sl * NCORES) + r * sl + (loc % sl)).astype(np.int64)


# ---------------------------------------------------------------------------
# device program
# ---------------------------------------------------------------------------
def build_program(cfg, c2_const, phases="abgc"):
    N, D, H1, D1, D2, K = cfg.N, cfg.D, cfg.H1, cfg.D1, cfg.D2, cfg.K
    NSH, NBLK, NSLICE = cfg.NSH, cfg.NBLK, cfg.NSLICE

    nc = bacc.Bacc("TRN2", target_bir_lowering=False, debug=False, num_devices=NCORES)

    xt = nc.dram_tensor("xt", [D, N], BF16, kind="ExternalInput")
    wpack1 = nc.dram_tensor("wpack1", [D, T1W], BF16, kind="ExternalInput")
    w2pack = nc.dram_tensor("w2pack", [D1, D2 + 2], BF16, kind="ExternalInput")
    b1p = nc.dram_tensor("b1p", [128, D1], F32, kind="ExternalInput")
    b2effr = nc.dram_tensor("b2effr", [128, D2], F32, kind="ExternalInput")
    iota = nc.dram_tensor("iota", [128, 128], F32, kind="ExternalInput")
    iotac = nc.dram_tensor("iotac", [128, 1], F32, kind="ExternalInput")
    iota_dk = nc.dram_tensor("iota_dk", [128, 128 * K], BF16, kind="ExternalInput")
    esrc1 = nc.dram_tensor("esrc1", [128, NBLK * K], I32, kind="ExternalInput")
    esrc2 = nc.dram_tensor("esrc2", [128, NBLK * K], I32, kind="ExternalInput")
    edstl = nc.dram_tensor("edstl", [128, NBLK * K], I32, kind="ExternalInput")
    edloc = nc.dram_tensor("edloc", [128, NBLK * K], BF16, kind="ExternalInput")
    out = nc.dram_tensor("out", [NSH, D2], F32, kind="ExternalOutput")

    dump = os.environ.get("KDUMP", "") == "1"
    t1 = nc.dram_tensor("t1", [N, T1W], BF16, kind="Internal")
    t2s = nc.dram_tensor("t2s", [NSH, T2W], BF16, kind="Internal")
    t2 = nc.dram_tensor("t2", [N, T2W], BF16, kind="Internal", addr_space="Shared")

    with tile.TileContext(nc) as tc:
        with tc.tile_pool(name="const", bufs=1) as cp:
            con = {}
            for name, hndl, dt in [
                ("wpack1", wpack1, BF16), ("w2pack", w2pack, BF16),
                ("b1p", b1p, F32), ("b2effr", b2effr, F32),
                ("iota", iota, F32), ("iotac", iotac, F32),
                ("iota_dk", iota_dk, BF16),
                ("esrc1", esrc1, I32), ("esrc2", esrc2, I32),
                ("edstl", edstl, I32), ("edloc", edloc, BF16),
            ]:
                t = cp.tile(list(hndl.shape), dt, tag=name)
                nc.sync.dma_start(out=t[:], in_=hndl[:])
                con[name] = t
            ident_bf = cp.tile([128, 128], BF16)
            nc.vector.tensor_tensor(
                out=ident_bf[:], in0=con["iotac"][:].to_broadcast([128, 128]),
                in1=con["iota"][:], op=AOP.is_equal,
            )
            con["ident_bf"] = ident_bf

            rep = int(os.environ.get("KREP", "1"))
            for r in range(rep):
                sfx = f"r{r}" if r else ""
                if "a" in phases:
                    _phase_a(nc, tc, cfg, xt, con["wpack1"], t1, sfx)
                if dump and "a" in phases and r == 0:
                    _dump(nc, tc, "t1dump", t1, nc.dram_tensor(
                        "t1dump", [N, T1W], BF16, kind="ExternalOutput"))
                if "b" in phases:
                    _edge_phase(nc, tc, cfg, 1, t1, t1, t2s,
                                t2 if "g" in phases else None, con, c2_const,
                                None, sfx)
                if dump and "b" in phases and r == 0:
                    _dump(nc, tc, "t2sdump", t2s, nc.dram_tensor(
                        "t2sdump", [NSH, T2W], BF16, kind="ExternalOutput"))
                if "c" in phases:
                    _edge_phase(nc, tc, cfg, 2, t2, t2s, t2s, None, con,
                                c2_const, out, sfx)

    nc.compile()
    return nc


def _dump(nc, tc, name, src, dst):
    rows, cols = src.shape
    with tc.tile_pool(name=name, bufs=2) as pd:
        for i in range(rows // 128):
            t = pd.tile([128, cols], BF16, tag="d")
            nc.sync.dma_start(out=t[:], in_=src[i * 128 : (i + 1) * 128, :])
            nc.sync.dma_start(out=dst[i * 128 : (i + 1) * 128, :], in_=t[:])


def _phase_a(nc, tc, cfg, xt, wpack1_t, t1, sfx=""):
    N = cfg.N
    ntile = N // 128
    GA = 8  # node tiles per group
    with (
        tc.tile_pool(name="pa_in" + sfx, bufs=3) as pin,
        tc.tile_pool(name="pa_ps" + sfx, bufs=4, space="PSUM") as pps,
        tc.tile_pool(name="pa_st" + sfx, bufs=3) as pst,
    ):
        for mt in range(ntile // GA):
            xt_t = pin.tile([128, 128 * GA], BF16, tag="xt")
            nc.sync.dma_start(
                out=xt_t[:], in_=xt[:, mt * 128 * GA : (mt + 1) * 128 * GA]
            )
            stg = pst.tile([128, GA * T1W], BF16, tag="stg")
            for half in range(2):
                ps = pps.tile([128, 4 * T1W], F32, tag="ps")
                for s in range(4):
                    st = half * 4 + s
                    nc.tensor.matmul(
                        out=ps[:, s * T1W : (s + 1) * T1W],
                        lhsT=xt_t[:, st * 128 : (st + 1) * 128],
                        rhs=wpack1_t[:], start=True, stop=True,
                    )
                # alternate copy engines so neither DVE nor ACT is the choke
                dstv = stg[:, half * 4 * T1W : (half + 1) * 4 * T1W]
                if half == 0:
                    nc.vector.tensor_copy(out=dstv, in_=ps[:])
                else:
                    nc.scalar.copy(out=dstv, in_=ps[:])
            dst_ap = bass.AP(
                t1[:].tensor,
                mt * 128 * GA * T1W,
                [[T1W, 128], [128 * T1W, GA], [1, T1W]],
            )
            nc.sync.dma_start(
                out=dst_ap, in_=stg[:].rearrange("p (s w) -> p s w", w=T1W)
            )


def _edge_phase(nc, tc, cfg, layer, vtab, atab, t2s, t2, con, c2_const, out,
                sfx=""):
    """layer 1: value/adst gathers from t1, produces t2s + sliced AllGather.
    layer 2: value gathers from t2, adst from t2s, produces out."""
    K, NBLK, NSLICE = cfg.K, cfg.NBLK, cfg.NSLICE
    BPS = NBLK // NSLICE
    SLN = cfg.NSH // NSLICE
    H1, C1, D1, D2 = cfg.H1, cfg.C1, cfg.D1, cfg.D2
    if layer == 1:
        TW, TV, H = T1W, T1V, H1
        RC = D1 + H1          # rhs cols: [v*w (c-major 64) | w (8)]
        aoff = T1V            # a_dst cols start in value table
    else:
        TW, TV, H = T2W, T2V, 1
        RC = D2 + 1           # rhs cols: [h2' | one] direct from gather
        aoff = T2V
    L = f"e{layer}" + sfx
    esrc = con["esrc1"] if layer == 1 else con["esrc2"]
    edstl = con["edstl"]
    edlocs = con["edloc"]
    iodk = con["iota_dk"]

    adst_t = os.environ.get("KADST", "g") == "t"  # t: PE-transpose, g: gathers
    with (
        tc.tile_pool(name=L + "_g", bufs=3) as pg,
        tc.tile_pool(name=L + "_a", bufs=2) as pa,
        tc.tile_pool(name=L + "_m", bufs=2) as pm,
        tc.tile_pool(name=L + "_r", bufs=2) as pr,
        tc.tile_pool(name=L + "_s", bufs=3) as psm,
        tc.tile_pool(name=L + "_acc", bufs=2, space="PSUM") as pacc,
        tc.tile_pool(name=L + "_mt", bufs=3, space="PSUM") as pmt,
        tc.tile_pool(name=L + "_ad", bufs=2, space="PSUM") as pad,
        tc.tile_pool(name=L + "_ep", bufs=2) as pep,
        tc.tile_pool(name=L + "_epp", bufs=2, space="PSUM") as pepp,
    ):
        for b in range(NBLK):
            # per-block value tile; chunk 0 = self-loops via direct DMA, the
            # rest one indirect row-gather per 128 edges (HW: 1 idx/partition)
            vg = pg.tile([128, K * TV], BF16, tag="vg")
            selftab = t2s if layer == 2 else vtab
            nc.sync.dma_start(
                out=vg[:, 0:TV], in_=selftab[b * BLK : (b + 1) * BLK, 0:TV]
            )
            for j in range(1, K):
                nc.gpsimd.indirect_dma_start(
                    out=vg[:, j * TV : (j + 1) * TV], out_offset=None, in_=vtab[:],
                    in_offset=bass.IndirectOffsetOnAxis(
                        ap=esrc[:, b * K + j : b * K + j + 1], axis=0),
                )

            # one-hot routing matrix M[e, d*K+j], all-packed bf16 -> 2x DVE
            m_t = pm.tile([128, 128 * K], BF16, tag="m")
            nc.vector.tensor_tensor(
                out=_ap(m_t[:], 0, [[K, 128], [1, K]]),
                in0=_ap(edlocs[:], b * K, [[0, 128], [1, K]]),
                in1=_ap(iodk[:], 0, [[K, 128], [1, K]]),
                op=AOP.is_equal,
            )

            # per-edge a_dst
            if adst_t:
                # adw window (own 128 dst rows) + M^T via PE transpose + matmul
                adw = pa.tile([128, H], BF16, tag="adw")
                nc.sync.dma_start(
                    out=adw[:],
                    in_=(t2s if layer == 2 else vtab)[
                        b * BLK : (b + 1) * BLK, aoff : aoff + H],
                )
                mts = pa.tile([128, 128 * K], BF16, tag="mts")
                TB = 8  # transpose chunks per PSUM bank tile
                for t0 in range(0, K, TB):
                    tn = min(TB, K - t0)
                    mtp = pmt.tile([128, 128 * TB], BF16, tag="mtp")
                    for j in range(t0, t0 + tn):
                        nc.tensor.transpose(
                            out=mtp[:, (j - t0) * 128 : (j - t0 + 1) * 128],
                            in_=_ap(m_t[:], j, [[K, 128]]),
                            identity=con["ident_bf"][:],
                        )
                    eng = nc.vector if (t0 // TB) % 2 == 0 else nc.scalar
                    if eng is nc.vector:
                        nc.vector.tensor_copy(
                            out=mts[:, t0 * 128 : (t0 + tn) * 128],
                            in_=mtp[:, 0 : tn * 128],
                        )
                    else:
                        nc.scalar.copy(
                            out=mts[:, t0 * 128 : (t0 + tn) * 128],
                            in_=mtp[:, 0 : tn * 128],
                        )
                adp = pad.tile([128, K * H], F32, tag="adp")
                for j in range(K):
                    nc.tensor.matmul(
                        out=adp[:, j * H : (j + 1) * H],
                        lhsT=mts[:, j * 128 : (j + 1) * 128],
                        rhs=adw[:], start=True, stop=True,
                    )
                ag_src = adp
            else:
                ag = pa.tile([128, K * H], BF16, tag="ag")
                nc.sync.dma_start(
                    out=ag[:, 0:H],
                    in_=(t2s if layer == 2 else vtab)[
                        b * BLK : (b + 1) * BLK, aoff : aoff + H],
                )
                for j in range(1, K):
                    nc.gpsimd.indirect_dma_start(
                        out=ag[:, j * H : (j + 1) * H], out_offset=None,
                        in_=atab[:],
                        in_offset=bass.IndirectOffsetOnAxis(
                            ap=edstl[:, b * K + j : b * K + j + 1], axis=0),
                        element_offset=aoff,
                    )
                ag_src = ag
            vo = 0
            ao = 0

            # logits: lg = asrc[src] + adst[dst]
            lg = psm.tile([128, K * H], BF16, tag="lg")
            if layer == 1:
                asrc_v = _ap(vg[:], vo + D1, [[TV, K], [1, H]])
                lg_o = _ap(lg[:], 0, [[H, K], [1, H]])
                ag_v = _ap(ag_src[:], ao, [[H, K], [1, H]])
            else:
                asrc_v = _ap(vg[:], vo + D2 + 1, [[TV, K]])
                lg_o = lg[:]
                ag_v = _ap(ag_src[:], ao, [[H, K]]) if adst_t else ag_src[:, ao : ao + K]
            nc.vector.tensor_tensor(out=lg_o, in0=asrc_v, in1=ag_v, op=AOP.add)
            lr = psm.tile([128, K * H], BF16, tag="lr")
            nc.vector.scalar_tensor_tensor(
                out=lr[:], in0=lg[:], scalar=NEG_SLOPE, in1=lg[:],
                op0=AOP.mult, op1=AOP.max,
            )

            if layer == 1:
                # rhs staging [v*w (c-major) | w]; w written by ACT exp
                rhs = pr.tile([128, K * RC], BF16, tag="rhs")
                w_v = _ap(rhs[:], D1, [[RC, K], [1, H]])
                nc.scalar.activation(
                    out=w_v, in_=_ap(lr[:], 0, [[H, K], [1, H]]), func=ACT.Exp
                )
                nc.vector.tensor_tensor(
                    out=_ap(rhs[:], 0, [[RC, K], [H, C1], [1, H]]),
                    in0=_ap(vg[:], vo, [[TV, K], [H, C1], [1, H]]),
                    in1=_ap(rhs[:], D1, [[RC, K], [0, C1], [1, H]]),
                    op=AOP.mult,
                )
                acc = pacc.tile([128, RC], F32, tag="acc")
                for j in range(K):
                    nc.tensor.matmul(
                        out=acc[:],
                        lhsT=_ap(m_t[:], j, [[K, 128]]),
                        rhs=rhs[:, j * RC : (j + 1) * RC],
                        start=(j == 0), stop=(j == K - 1),
                    )
            else:
                w_t = psm.tile([128, K], BF16, tag="w")
                nc.scalar.activation(out=w_t[:], in_=lr[:], func=ACT.Exp)
                mw = pm.tile([128, 128 * K], BF16, tag="mw")
                nc.vector.tensor_tensor(
                    out=_ap(mw[:], 0, [[K, 128], [1, K]]),
                    in0=_ap(m_t[:], 0, [[K, 128], [1, K]]),
                    in1=_ap(w_t[:], 0, [[0, 128], [1, K]]),
                    op=AOP.mult,
                )
                acc = pacc.tile([128, RC], F32, tag="acc")
                for j in range(K):
                    nc.tensor.matmul(
                        out=acc[:],
                        lhsT=_ap(mw[:], j, [[K, 128]]),
                        rhs=vg[:, vo + j * TV : vo + j * TV + RC],
                        start=(j == 0), stop=(j == K - 1),
                    )

            # ---------------- block epilogue ------------------------------
            accs = pep.tile([128, RC], F32, tag="accs")
            nc.scalar.copy(out=accs[:], in_=acc[:])
            if layer == 2:
                sinv = pep.tile([128, 1], F32, tag="sinv")
                nc.vector.reciprocal(out=sinv[:], in_=accs[:, D2 : D2 + 1])
                o1 = pep.tile([128, D2], F32, tag="o1")
                nc.scalar.activation(
                    out=o1[:], in_=accs[:, 0:D2], func=ACT.Copy, scale=sinv[:]
                )
                o2 = pep.tile([128, D2], F32, tag="o2")
                nc.vector.tensor_add(out=o2[:], in0=o1[:], in1=con["b2effr"][:])
                nc.sync.dma_start(out=out[b * BLK : (b + 1) * BLK, :], in_=o2[:])
                continue

            # L1: y = (acc_v / acc_w per head) + b1, helu = elu(y)+1, h2' = helu@W2
            sinv = pep.tile([128, H1], F32, tag="sinv")
            nc.vector.reciprocal(out=sinv[:], in_=accs[:, D1 : D1 + H1])
            y = pep.tile([128, D1], F32, tag="y")
            nc.vector.tensor_tensor(
                out=_ap(y[:], 0, [[H1, C1], [1, H1]]),
                in0=_ap(accs[:], 0, [[H1, C1], [1, H1]]),
                in1=_ap(sinv[:], 0, [[0, C1], [1, H1]]),
                op=AOP.mult,
            )
            nc.vector.tensor_add(out=y[:], in0=y[:], in1=con["b1p"][:])
            tmin = pep.tile([128, D1], F32, tag="tmin")
            nc.vector.tensor_scalar_min(out=tmin[:], in0=y[:], scalar1=0.0)
            e_t = pep.tile([128, D1], F32, tag="e")
            nc.scalar.activation(out=e_t[:], in_=tmin[:], func=ACT.Exp)
            helu = pep.tile([128, D1], F32, tag="helu")
            nc.vector.scalar_tensor_tensor(
                out=helu[:], in0=y[:], scalar=0.0, in1=e_t[:],
                op0=AOP.max, op1=AOP.add,
            )
            # center: elu(y) = helu - 1 exactly in f32, THEN quantize — keeps
            # the t2 table free of the colsum(W2) cancellation
            hcent = pep.tile([128, D1], BF16, tag="hcent")
            nc.vector.tensor_scalar_add(out=hcent[:], in0=helu[:], scalar1=-1.0)
            htp = pepp.tile([D1, 128], BF16, tag="htp")
            nc.tensor.transpose(out=htp[:], in_=hcent[:], identity=con["ident_bf"][:])
            hts = pep.tile([D1, 128], BF16, tag="hts")
            nc.vector.tensor_copy(out=hts[:], in_=htp[:])
            h2p = pepp.tile([128, D2 + 2], F32, tag="h2p")
            nc.tensor.matmul(
                out=h2p[:], lhsT=hts[:], rhs=con["w2pack"][:], start=True, stop=True
            )
            stg = pep.tile([128, T2W], BF16, tag="stg")
            nc.scalar.copy(out=stg[:, 0:D2], in_=h2p[:, 0:D2])
            nc.vector.memset(stg[:, D2 : D2 + 1], 1.0)
            nc.vector.tensor_copy(
                out=stg[:, D2 + 1 : D2 + 3], in_=h2p[:, D2 : D2 + 2]
            )
            nc.vector.memset(stg[:, D2 + 3 : T2W], 0.0)
            nc.sync.dma_start(out=t2s[b * BLK : (b + 1) * BLK, :], in_=stg[:])

            if t2 is not None and (b + 1) % BPS == 0:
                s = (b + 1) // BPS - 1
                nc.gpsimd.collective_compute(
                    "AllGather",
                    AOP.bypass,
                    replica_groups=[list(range(NCORES))],
                    ins=[t2s[s * SLN : (s + 1) * SLN, :]],
                    outs=[t2[s * SLN * NCORES : (s + 1) * SLN * NCORES, :]],
                )


# ---------------------------------------------------------------------------
# host glue
# ---------------------------------------------------------------------------
def prepare(x, seq, edges, W1, att_src1, att_dst1, b1, W2, att_src2,
            att_dst2, b2, nslice=4):
    import ml_dtypes

    bf = ml_dtypes.bfloat16
    nb, ncn, d = x.shape
    N = nb * ncn
    H1, C1 = att_src1.shape
    D1 = H1 * C1
    D2 = W2.shape[1]

    xf = (np.asarray(x, np.float32).reshape(N, d)
          * np.asarray(seq, np.float32).reshape(N, 1))
    src = np.asarray(edges[0], np.int64)
    dst = np.asarray(edges[1], np.int64)
    k, esrc_g, dloc = _edge_schedule(src, dst, N)
    cfg = Cfg(N, d, H1, C1, D2, k, nslice)

    # (c-major) head permutation: new col c*H1+h  <-  old col h*C1+c
    new2old = np.empty(D1, dtype=np.int64)
    for c in range(C1):
        for h in range(H1):
            new2old[c * H1 + h] = h * C1 + c
    w1 = np.asarray(W1, np.float32)
    wsrc = np.einsum("khc,hc->kh", w1.reshape(d, H1, C1), np.asarray(att_src1, np.float32))
    wdst = np.einsum("khc,hc->kh", w1.reshape(d, H1, C1), np.asarray(att_dst1, np.float32))
    wpack1 = np.concatenate([w1[:, new2old], wsrc, wdst], axis=1).astype(bf)

    w2a = np.asarray(W2, np.float32)
    a2s = np.asarray(att_src2, np.float32).reshape(-1)
    a2d = np.asarray(att_dst2, np.float32).reshape(-1)
    c2_const = 0.0  # t2 table is centered on-device; no constant needed
    b2eff = np.asarray(b2, np.float32)

    w2p = w2a[new2old, :]
    w2pack = np.concatenate(
        [w2p, (w2p @ a2s)[:, None], (w2p @ a2d)[:, None]], axis=1).astype(bf)

    b1p = np.tile(np.asarray(b1, np.float32)[new2old][None, :], (128, 1)).astype(np.float32)
    b2effr = np.tile(b2eff[None, :], (128, 1)).astype(np.float32)
    iota = np.tile(np.arange(128, dtype=np.float32)[None, :], (128, 1))
    iotac = np.arange(128, dtype=np.float32)[:, None].copy()
    iota_dk = np.repeat(np.arange(128, dtype=np.float32), k)[None, :]
    iota_dk = np.tile(iota_dk, (128, 1)).astype(bf)

    nblk = cfg.NBLK
    # local dst row per slot: b*128 + dloc (0 for pads)
    dloc_cb = dloc  # [NCORES, nblk, 128, k], -1 for pads
    base = (np.arange(nblk, dtype=np.float32) * BLK)[None, :, None, None]
    edst_local = np.where(dloc_cb >= 0, dloc_cb + base, 0.0).astype(np.int64)

    def to_sb(a, dt):
        # [nblk, 128, k] -> [128, nblk*k]
        return np.ascontiguousarray(
            a.transpose(1, 0, 2).reshape(128, nblk * k)).astype(dt)

    phys = _t2_phys(cfg)
    in_maps = []
    for c in range(NCORES):
        rot = (np.arange(N, dtype=np.int64) + c * cfg.NSH) % N
        xt_c = np.ascontiguousarray(xf[rot].T).astype(bf)
        e1 = ((esrc_g[c] - c * cfg.NSH) % N)
        e2 = phys[esrc_g[c]]
        in_maps.append(
            {
                "xt": xt_c,
                "wpack1": wpack1,
                "w2pack": w2pack,
                "b1p": b1p,
                "b2effr": b2effr,
                "iota": iota,
                "iotac": iotac,
                "iota_dk": iota_dk,
                "esrc1": to_sb(e1, np.int32),
                "esrc2": to_sb(e2, np.int32),
                "edstl": to_sb(edst_local[c], np.int32),
                "edloc": to_sb(dloc_cb[c], bf),
            }
        )
    return cfg, c2_const, in_maps


_CACHE = {}
LAST_RESULT = None


def kernel(**inputs) -> np.ndarray:
    from concourse.bass_utils import run_bass_kernel_spmd

    global LAST_RESULT
    x = np.asarray(inputs["x"])
    nb, ncn, d = x.shape
    nslice = int(os.environ.get("KNSLICE", "4"))
    cfg, c2_const, in_maps = prepare(**{k: inputs[k] for k in (
        "x", "seq", "edges", "W1", "att_src1", "att_dst1", "b1",
        "W2", "att_src2", "att_dst2", "b2")}, nslice=nslice)

    phases = os.environ.get("KPHASES", "abgc")
    key = (cfg.N, cfg.D, cfg.H1, cfg.C1, cfg.D2, cfg.K, cfg.NSLICE,
           round(c2_const, 10), phases)
    if key not in _CACHE:
        _CACHE.clear()
        _CACHE[key] = build_program(cfg, c2_const, phases=phases)
    nc = _CACHE[key]

    res = run_bass_kernel_spmd(nc, in_maps, core_ids=list(range(NCORES)), trace=False)
    LAST_RESULT = res
    shards = [res.results[c]["out"] for c in range(NCORES)]
    full = np.concatenate(shards, axis=0)
    return full.reshape(nb, ncn, d).astype(np.float32)


# revision 26
# speedup vs baseline: 2.3270x; 2.3270x over previous
"""2-layer GAT (PyG GATConv semantics) on 8 Trainium2 NeuronCores via Bass/Tile.

Contract: kernel(**inputs) takes the FULL inputs of reference.setup_inputs()
and returns the FULL [16, 4096, 128] float32 output.

v2 strategy (dst-node sharding, one SPMD program), redesigned from the v1
baseline around the measured bottlenecks (Pool/SWDGE fixed cost per indirect
DMA, DVE fixed cost per small op, f32 table bytes):

- Tables are bf16. t1 row (80 cols, 160B): [h1 (c-major: col c*H+h) | asrc1
  (H) | adst1 (H)]. t2 row (132 cols): [h2' | 1.0 | asrc2'(-c2 folded) |
  adst2' | pad]. The constant 1.0 column turns the softmax denominator into
  ordinary matmul columns.
- Per GB-block group, ONE batched indirect DMA gathers all GB*K*128 value
  rows ([h|asrc] cols only) and ONE more gathers per-edge a_dst[dst] (the
  dst rows' a_dst cols via element_offset). This cuts SWDGE generation from
  one ~1us Pool occupancy per 128 edges to one per GB*K*128 edges.
- Per-edge math is batched across the whole block on DVE in a handful of
  16-bit 2x-rate ops: the one-hot routing matrix M[e, d*K+j] (K-innermost
  layout against a host-built iota_dK constant so every operand is packed
  2-byte), logits lg = asrc[src]+adst[dst], lrelu, and the w-fold
  (L1: vf = h*w per head with c-major layout so the head-broadcast is
  packed; L2: Mw = M*w).
- Aggregation per 128-dst block: PSUM acc[d, :] += M_j^T @ rhs_j with
  rhs_j = [v*w | w] (L1) / Mw_j^T @ [h2' | 1] (L2). Softmax max-subtraction
  is skipped (logits are O(1); exact in f32 accumulation).
- ELU's -1 is folded out: t2 stores h2' = (elu+1)@W2; since sum(alpha)=1 the
  colsum(W2) correction folds into the output bias and the L2 logit constant.
- t2 shards AllGather in NSLICE slices overlapped with phase-B compute.
"""

import os
import sys

import numpy as np

if "/opt/trn_rl_repo" not in sys.path:
    sys.path.insert(0, "/opt/trn_rl_repo")

import concourse.bass as bass
import concourse.bacc as bacc
import concourse.mybir as mybir
import concourse.tile as tile

F32 = mybir.dt.float32
BF16 = mybir.dt.bfloat16
I32 = mybir.dt.int32
AOP = mybir.AluOpType
ACT = mybir.ActivationFunctionType

NEG_SLOPE = 0.2
NCORES = 8
BLK = 128

T1W = 80    # t1 row cols: [0:64] h1 (c-major), [64:72] asrc1, [72:80] adst1
T1V = 72    # value-gather cols of t1
T2W = 132   # t2 row cols: [0:128] h2', [128] one, [129] asrc2', [130] adst2', [131] pad
T2V = 130   # value-gather cols of t2
GB = 4      # blocks per batched gather instruction


class Cfg:
    def __init__(self, n_nodes, d_in, h1, c1, d2, k, nslice):
        self.N = n_nodes
        self.D = d_in
        self.H1 = h1
        self.C1 = c1
        self.D1 = h1 * c1
        self.D2 = d2
        self.K = k
        self.NSH = n_nodes // NCORES
        self.NBLK = self.NSH // BLK
        self.NSLICE = nslice
        assert self.NSH % BLK == 0 and self.NBLK % nslice == 0
        assert self.NBLK % GB == 0 and (self.NBLK // nslice) % GB == 0


def _ap(t_ap, off, dims):
    """Raw AP view of a tile slice: partition dim kept, free dims replaced.
    `off` in elements, `dims` = [[stride, size], ...]."""
    a = [list(p) for p in t_ap.ap]
    return bass.AP(t_ap.tensor, t_ap.offset + off, [a[0]] + dims)


# ---------------------------------------------------------------------------
# host-side edge schedule
# ---------------------------------------------------------------------------
def _edge_schedule(src, dst, n_nodes):
    """src/dst int64 arrays (random edges only; self-loops get a dedicated
    chunk 0 served by direct DMA). Slot (p, j) for j>=1 = random edge; the
    returned arrays are [NCORES, nblk, 128, K] with chunk 0 = self-loops."""
    nsh = n_nodes // NCORES
    nblk = nsh // BLK
    order = np.argsort(dst, kind="stable")
    src = src[order]
    dst = dst[order]
    blk_of = dst // BLK
    nblk_g = n_nodes // BLK
    counts = np.bincount(blk_of, minlength=nblk_g)
    kr = int((int(counts.max()) + 127) // 128)
    k = kr + 1
    starts = np.zeros(nblk_g + 1, dtype=np.int64)
    np.cumsum(counts, out=starts[1:])

    slots = 128 * kr
    esrc = np.zeros((NCORES, nblk, 128, k), dtype=np.int64)
    edloc = np.full((NCORES, nblk, 128, k), -1.0, dtype=np.float32)
    # chunk 0: self-loops (value/adst via direct DMA; edloc drives M)
    node = np.arange(n_nodes, dtype=np.int64).reshape(NCORES, nblk, 128)
    esrc[:, :, :, 0] = node
    edloc[:, :, :, 0] = np.tile(np.arange(128, dtype=np.float32), (NCORES, nblk, 1))
    for g in range(nblk_g):
        c, b = divmod(g, nblk)
        s0, s1 = int(starts[g]), int(starts[g + 1])
        n = s1 - s0
        flat_src = np.zeros(slots, dtype=np.int64)
        flat_loc = np.full(slots, -1.0, dtype=np.float32)
        flat_src[:n] = src[s0:s1]
        flat_loc[:n] = (dst[s0:s1] - g * BLK).astype(np.float32)
        esrc[c, b, :, 1:] = flat_src.reshape(128, kr)
        edloc[c, b, :, 1:] = flat_loc.reshape(128, kr)
    return k, esrc, edloc


def _t2_phys(cfg):
    """node id -> physical t2 row (slice-major: slice, rank, local)."""
    N, NSH, NSLICE = cfg.N, cfg.NSH, cfg.NSLICE
    sl = NSH // NSLICE
    node = np.arange(N, dtype=np.int64)
    r = node // NSH
    loc = node % NSH
    s = loc // sl
    return (s * (sl * NCORES) + r * sl + (loc % sl)).astype(np.int64)


# ---------------------------------------------------------------------------
# device program
# ---------------------------------------------------------------------------
def build_program(cfg, c2_const, phases="abgc"):
    N, D, H1, D1, D2, K = cfg.N, cfg.D, cfg.H1, cfg.D1, cfg.D2, cfg.K
    NSH, NBLK, NSLICE = cfg.NSH, cfg.NBLK, cfg.NSLICE

    nqueues = 4 if os.environ.get("KQRR", "1") == "1" else 1
    nc = bacc.Bacc("TRN2", target_bir_lowering=False, debug=False,
                   num_devices=NCORES, num_swdge_queues=nqueues)

    xt = nc.dram_tensor("xt", [D, N], BF16, kind="ExternalInput")
    wpack1 = nc.dram_tensor("wpack1", [D, T1W], BF16, kind="ExternalInput")
    w2pack = nc.dram_tensor("w2pack", [D1, D2 + 2], BF16, kind="ExternalInput")
    b1p = nc.dram_tensor("b1p", [128, D1], F32, kind="ExternalInput")
    b2effr = nc.dram_tensor("b2effr", [128, D2], F32, kind="ExternalInput")
    iota = nc.dram_tensor("iota", [128, 128], F32, kind="ExternalInput")
    iotac = nc.dram_tensor("iotac", [128, 1], F32, kind="ExternalInput")
    iota_dk = nc.dram_tensor("iota_dk", [128, 128 * K], BF16, kind="ExternalInput")
    esrc1 = nc.dram_tensor("esrc1", [128, NBLK * K], I32, kind="ExternalInput")
    esrc2 = nc.dram_tensor("esrc2", [128, NBLK * K], I32, kind="ExternalInput")
    edstl = nc.dram_tensor("edstl", [128, NBLK * K], I32, kind="ExternalInput")
    edloc = nc.dram_tensor("edloc", [128, NBLK * K], BF16, kind="ExternalInput")
    out = nc.dram_tensor("out", [NSH, D2], F32, kind="ExternalOutput")

    dump = os.environ.get("KDUMP", "") == "1"
    t1 = nc.dram_tensor("t1", [N, T1W], BF16, kind="Internal")
    t2s = nc.dram_tensor("t2s", [NSH, T2W], BF16, kind="Internal")
    t2 = nc.dram_tensor("t2", [N, T2W], BF16, kind="Internal", addr_space="Shared")

    with tile.TileContext(nc) as tc:
        with tc.tile_pool(name="const", bufs=1) as cp:
            con = {}
            for name, hndl, dt in [
                ("wpack1", wpack1, BF16), ("w2pack", w2pack, BF16),
                ("b1p", b1p, F32), ("b2effr", b2effr, F32),
                ("iota", iota, F32), ("iotac", iotac, F32),
                ("iota_dk", iota_dk, BF16),
                ("esrc1", esrc1, I32), ("esrc2", esrc2, I32),
                ("edstl", edstl, I32), ("edloc", edloc, BF16),
            ]:
                t = cp.tile(list(hndl.shape), dt, tag=name)
                nc.sync.dma_start(out=t[:], in_=hndl[:])
                con[name] = t
            ident_bf = cp.tile([128, 128], BF16)
            nc.vector.tensor_tensor(
                out=ident_bf[:], in0=con["iotac"][:].to_broadcast([128, 128]),
                in1=con["iota"][:], op=AOP.is_equal,
            )
            con["ident_bf"] = ident_bf

            rep = int(os.environ.get("KREP", "1"))
            for r in range(rep):
                sfx = f"r{r}" if r else ""
                if "a" in phases:
                    _phase_a(nc, tc, cfg, xt, con["wpack1"], t1, sfx)
                if dump and "a" in phases and r == 0:
                    _dump(nc, tc, "t1dump", t1, nc.dram_tensor(
                        "t1dump", [N, T1W], BF16, kind="ExternalOutput"))
                if "b" in phases:
                    _edge_phase(nc, tc, cfg, 1, t1, t1, t2s,
                                t2 if "g" in phases else None, con, c2_const,
                                None, sfx)
                if dump and "b" in phases and r == 0:
                    _dump(nc, tc, "t2sdump", t2s, nc.dram_tensor(
                        "t2sdump", [NSH, T2W], BF16, kind="ExternalOutput"))
                if "c" in phases:
                    _edge_phase(nc, tc, cfg, 2, t2, t2s, t2s, None, con,
                                c2_const, out, sfx)

    nc.compile()
    return nc


def _dump(nc, tc, name, src, dst):
    rows, cols = src.shape
    with tc.tile_pool(name=name, bufs=2) as pd:
        for i in range(rows // 128):
            t = pd.tile([128, cols], BF16, tag="d")
            nc.sync.dma_start(out=t[:], in_=src[i * 128 : (i + 1) * 128, :])
            nc.sync.dma_start(out=dst[i * 128 : (i + 1) * 128, :], in_=t[:])


def _phase_a(nc, tc, cfg, xt, wpack1_t, t1, sfx=""):
    N = cfg.N
    ntile = N // 128
    GA = 8  # node tiles per group
    with (
        tc.tile_pool(name="pa_in" + sfx, bufs=3) as pin,
        tc.tile_pool(name="pa_ps" + sfx, bufs=4, space="PSUM") as pps,
        tc.tile_pool(name="pa_st" + sfx, bufs=3) as pst,
    ):
        for mt in range(ntile // GA):
            xt_t = pin.tile([128, 128 * GA], BF16, tag="xt")
            nc.sync.dma_start(
                out=xt_t[:], in_=xt[:, mt * 128 * GA : (mt + 1) * 128 * GA]
            )
            stg = pst.tile([128, GA * T1W], BF16, tag="stg")
            for half in range(2):
                ps = pps.tile([128, 4 * T1W], F32, tag="ps")
                for s in range(4):
                    st = half * 4 + s
                    nc.tensor.matmul(
                        out=ps[:, s * T1W : (s + 1) * T1W],
                        lhsT=xt_t[:, st * 128 : (st + 1) * 128],
                        rhs=wpack1_t[:], start=True, stop=True,
                    )
                # alternate copy engines so neither DVE nor ACT is the choke
                dstv = stg[:, half * 4 * T1W : (half + 1) * 4 * T1W]
                if half == 0:
                    nc.vector.tensor_copy(out=dstv, in_=ps[:])
                else:
                    nc.scalar.copy(out=dstv, in_=ps[:])
            dst_ap = bass.AP(
                t1[:].tensor,
                mt * 128 * GA * T1W,
                [[T1W, 128], [128 * T1W, GA], [1, T1W]],
            )
            nc.sync.dma_start(
                out=dst_ap, in_=stg[:].rearrange("p (s w) -> p s w", w=T1W)
            )


def _edge_phase(nc, tc, cfg, layer, vtab, atab, t2s, t2, con, c2_const, out,
                sfx=""):
    """layer 1: value/adst gathers from t1, produces t2s + sliced AllGather.
    layer 2: value gathers from t2, adst from t2s, produces out."""
    K, NBLK, NSLICE = cfg.K, cfg.NBLK, cfg.NSLICE
    BPS = NBLK // NSLICE
    SLN = cfg.NSH // NSLICE
    H1, C1, D1, D2 = cfg.H1, cfg.C1, cfg.D1, cfg.D2
    if layer == 1:
        TW, TV, H = T1W, T1V, H1
        RC = D1 + H1          # rhs cols: [v*w (c-major 64) | w (8)]
        aoff = T1V            # a_dst cols start in value table
    else:
        TW, TV, H = T2W, T2V, 1
        RC = D2 + 1           # rhs cols: [h2' | one] direct from gather
        aoff = T2V
    L = f"e{layer}" + sfx
    esrc = con["esrc1"] if layer == 1 else con["esrc2"]
    edstl = con["edstl"]
    edlocs = con["edloc"]
    iodk = con["iota_dk"]

    adst_t = os.environ.get("KADST", "t") == "t"  # t: PE-transpose, g: gathers
    with (
        tc.tile_pool(name=L + "_g", bufs=3) as pg,
        tc.tile_pool(name=L + "_a", bufs=2) as pa,
        tc.tile_pool(name=L + "_m", bufs=2) as pm,
        tc.tile_pool(name=L + "_r", bufs=2) as pr,
        tc.tile_pool(name=L + "_s", bufs=3) as psm,
        tc.tile_pool(name=L + "_acc", bufs=2, space="PSUM") as pacc,
        tc.tile_pool(name=L + "_mt", bufs=2, space="PSUM") as pmt,
        tc.tile_pool(name=L + "_ad", bufs=1, space="PSUM") as pad,
        tc.tile_pool(name=L + "_ep", bufs=2) as pep,
        tc.tile_pool(name=L + "_epp", bufs=1, space="PSUM") as pepp,
    ):
        for b in range(NBLK):
            # per-block value tile; chunk 0 = self-loops via direct DMA, the
            # rest one indirect row-gather per 128 edges (HW: 1 idx/partition)
            vg = pg.tile([128, K * TV], BF16, tag="vg")
            selftab = t2s if layer == 2 else vtab
            nc.sync.dma_start(
                out=vg[:, 0:TV], in_=selftab[b * BLK : (b + 1) * BLK, 0:TV]
            )
            nq = nc.num_swdge_queues
            for j in range(1, K):
                ins = nc.gpsimd.indirect_dma_start(
                    out=vg[:, j * TV : (j + 1) * TV], out_offset=None, in_=vtab[:],
                    in_offset=bass.IndirectOffsetOnAxis(
                        ap=esrc[:, b * K + j : b * K + j + 1], axis=0),
                )
                if nq > 1:
                    q = j % nq
                    ins.ins.queue = f"qPoolDynamic{q or ''}"

            # one-hot routing matrix M[e, d*K+j], all-packed bf16 -> 2x DVE
            m_t = pm.tile([128, 128 * K], BF16, tag="m")
            nc.vector.tensor_tensor(
                out=_ap(m_t[:], 0, [[K, 128], [1, K]]),
                in0=_ap(edlocs[:], b * K, [[0, 128], [1, K]]),
                in1=_ap(iodk[:], 0, [[K, 128], [1, K]]),
                op=AOP.is_equal,
            )

            # per-edge a_dst
            if adst_t:
                # adw window (own 128 dst rows) + M^T via PE transpose + matmul
                adw = pa.tile([128, H], BF16, tag="adw")
                nc.sync.dma_start(
                    out=adw[:],
                    in_=(t2s if layer == 2 else vtab)[
                        b * BLK : (b + 1) * BLK, aoff : aoff + H],
                )
                mts = pa.tile([128, 128 * K], BF16, tag="mts")
                TB = 8  # transpose chunks per PSUM bank tile
                for t0 in range(0, K, TB):
                    tn = min(TB, K - t0)
                    mtp = pmt.tile([128, 128 * TB], BF16, tag="mtp")
                    for j in range(t0, t0 + tn):
                        nc.tensor.transpose(
                            out=mtp[:, (j - t0) * 128 : (j - t0 + 1) * 128],
                            in_=_ap(m_t[:], j, [[K, 128]]),
                            identity=con["ident_bf"][:],
                        )
                    eng = nc.vector if (t0 // TB) % 2 == 0 else nc.scalar
                    if eng is nc.vector:
                        nc.vector.tensor_copy(
                            out=mts[:, t0 * 128 : (t0 + tn) * 128],
                            in_=mtp[:, 0 : tn * 128],
                        )
                    else:
                        nc.scalar.copy(
                            out=mts[:, t0 * 128 : (t0 + tn) * 128],
                            in_=mtp[:, 0 : tn * 128],
                        )
                adp = pad.tile([128, K * H], F32, tag="adp")
                for j in range(K):
                    nc.tensor.matmul(
                        out=adp[:, j * H : (j + 1) * H],
                        lhsT=mts[:, j * 128 : (j + 1) * 128],
                        rhs=adw[:], start=True, stop=True,
                    )
                ag_src = adp
            else:
                ag = pa.tile([128, K * H], BF16, tag="ag")
                nc.sync.dma_start(
                    out=ag[:, 0:H],
                    in_=(t2s if layer == 2 else vtab)[
                        b * BLK : (b + 1) * BLK, aoff : aoff + H],
                )
                for j in range(1, K):
                    nc.gpsimd.indirect_dma_start(
                        out=ag[:, j * H : (j + 1) * H], out_offset=None,
                        in_=atab[:],
                        in_offset=bass.IndirectOffsetOnAxis(
                            ap=edstl[:, b * K + j : b * K + j + 1], axis=0),
                        element_offset=aoff,
                    )
                ag_src = ag
            vo = 0
            ao = 0

            # logits: lg = asrc[src] + adst[dst]
            lg = psm.tile([128, K * H], BF16, tag="lg")
            if layer == 1:
                asrc_v = _ap(vg[:], vo + D1, [[TV, K], [1, H]])
                lg_o = _ap(lg[:], 0, [[H, K], [1, H]])
                ag_v = _ap(ag_src[:], ao, [[H, K], [1, H]])
            else:
                asrc_v = _ap(vg[:], vo + D2 + 1, [[TV, K]])
                lg_o = lg[:]
                ag_v = _ap(ag_src[:], ao, [[H, K]]) if adst_t else ag_src[:, ao : ao + K]
            nc.vector.tensor_tensor(out=lg_o, in0=asrc_v, in1=ag_v, op=AOP.add)
            lr = psm.tile([128, K * H], BF16, tag="lr")
            nc.vector.scalar_tensor_tensor(
                out=lr[:], in0=lg[:], scalar=NEG_SLOPE, in1=lg[:],
                op0=AOP.mult, op1=AOP.max,
            )

            if layer == 1:
                # rhs staging [v*w (c-major) | w]; w written by ACT exp
                rhs = pr.tile([128, K * RC], BF16, tag="rhs")
                w_v = _ap(rhs[:], D1, [[RC, K], [1, H]])
                nc.scalar.activation(
                    out=w_v, in_=_ap(lr[:], 0, [[H, K], [1, H]]), func=ACT.Exp
                )
                nc.vector.tensor_tensor(
                    out=_ap(rhs[:], 0, [[RC, K], [H, C1], [1, H]]),
                    in0=_ap(vg[:], vo, [[TV, K], [H, C1], [1, H]]),
                    in1=_ap(rhs[:], D1, [[RC, K], [0, C1], [1, H]]),
                    op=AOP.mult,
                )
                acc = pacc.tile([128, RC], F32, tag="acc")
                for j in range(K):
                    nc.tensor.matmul(
                        out=acc[:],
                        lhsT=_ap(m_t[:], j, [[K, 128]]),
                        rhs=rhs[:, j * RC : (j + 1) * RC],
                        start=(j == 0), stop=(j == K - 1),
                    )
            else:
                w_t = psm.tile([128, K], BF16, tag="w")
                nc.scalar.activation(out=w_t[:], in_=lr[:], func=ACT.Exp)
                mw = pm.tile([128, 128 * K], BF16, tag="mw")
                nc.vector.tensor_tensor(
                    out=_ap(mw[:], 0, [[K, 128], [1, K]]),
                    in0=_ap(m_t[:], 0, [[K, 128], [1, K]]),
                    in1=_ap(w_t[:], 0, [[0, 128], [1, K]]),
                    op=AOP.mult,
                )
                acc = pacc.tile([128, RC], F32, tag="acc")
                for j in range(K):
                    nc.tensor.matmul(
                        out=acc[:],
                        lhsT=_ap(mw[:], j, [[K, 128]]),
                        rhs=vg[:, vo + j * TV : vo + j * TV + RC],
                        start=(j == 0), stop=(j == K - 1),
                    )

            # ---------------- block epilogue ------------------------------
            accs = pep.tile([128, RC], F32, tag="accs")
            nc.scalar.copy(out=accs[:], in_=acc[:])
            if layer == 2:
                sinv = pep.tile([128, 1], F32, tag="sinv")
                nc.vector.reciprocal(out=sinv[:], in_=accs[:, D2 : D2 + 1])
                o1 = pep.tile([128, D2], F32, tag="o1")
                nc.scalar.activation(
                    out=o1[:], in_=accs[:, 0:D2], func=ACT.Copy, scale=sinv[:]
                )
                o2 = pep.tile([128, D2], F32, tag="o2")
                nc.vector.tensor_add(out=o2[:], in0=o1[:], in1=con["b2effr"][:])
                nc.sync.dma_start(out=out[b * BLK : (b + 1) * BLK, :], in_=o2[:])
                continue

            # L1: y = (acc_v / acc_w per head) + b1, helu = elu(y)+1, h2' = helu@W2
            sinv = pep.tile([128, H1], F32, tag="sinv")
            nc.vector.reciprocal(out=sinv[:], in_=accs[:, D1 : D1 + H1])
            y = pep.tile([128, D1], F32, tag="y")
            nc.vector.tensor_tensor(
                out=_ap(y[:], 0, [[H1, C1], [1, H1]]),
                in0=_ap(accs[:], 0, [[H1, C1], [1, H1]]),
                in1=_ap(sinv[:], 0, [[0, C1], [1, H1]]),
                op=AOP.mult,
            )
            nc.vector.tensor_add(out=y[:], in0=y[:], in1=con["b1p"][:])
            tmin = pep.tile([128, D1], F32, tag="tmin")
            nc.vector.tensor_scalar_min(out=tmin[:], in0=y[:], scalar1=0.0)
            e_t = pep.tile([128, D1], F32, tag="e")
            nc.scalar.activation(out=e_t[:], in_=tmin[:], func=ACT.Exp)
            helu = pep.tile([128, D1], F32, tag="helu")
            nc.vector.scalar_tensor_tensor(
                out=helu[:], in0=y[:], scalar=0.0, in1=e_t[:],
                op0=AOP.max, op1=AOP.add,
            )
            # center: elu(y) = helu - 1 exactly in f32, THEN quantize — keeps
            # the t2 table free of the colsum(W2) cancellation
            hcent = pep.tile([128, D1], BF16, tag="hcent")
            nc.vector.tensor_scalar_add(out=hcent[:], in0=helu[:], scalar1=-1.0)
            htp = pepp.tile([D1, 128], BF16, tag="htp")
            nc.tensor.transpose(out=htp[:], in_=hcent[:], identity=con["ident_bf"][:])
            hts = pep.tile([D1, 128], BF16, tag="hts")
            nc.vector.tensor_copy(out=hts[:], in_=htp[:])
            h2p = pepp.tile([128, D2 + 2], F32, tag="h2p")
            nc.tensor.matmul(
                out=h2p[:], lhsT=hts[:], rhs=con["w2pack"][:], start=True, stop=True
            )
            stg = pep.tile([128, T2W], BF16, tag="stg")
            nc.scalar.copy(out=stg[:, 0:D2], in_=h2p[:, 0:D2])
            nc.vector.memset(stg[:, D2 : D2 + 1], 1.0)
            nc.vector.tensor_copy(
                out=stg[:, D2 + 1 : D2 + 3], in_=h2p[:, D2 : D2 + 2]
            )
            nc.vector.memset(stg[:, D2 + 3 : T2W], 0.0)
            nc.sync.dma_start(out=t2s[b * BLK : (b + 1) * BLK, :], in_=stg[:])

            if t2 is not None and (b + 1) % BPS == 0:
                s = (b + 1) // BPS - 1
                nc.gpsimd.collective_compute(
                    "AllGather",
                    AOP.bypass,
                    replica_groups=[list(range(NCORES))],
                    ins=[t2s[s * SLN : (s + 1) * SLN, :]],
                    outs=[t2[s * SLN * NCORES : (s + 1) * SLN * NCORES, :]],
                )


# ---------------------------------------------------------------------------
# host glue
# ---------------------------------------------------------------------------
def prepare(x, seq, edges, W1, att_src1, att_dst1, b1, W2, att_src2,
            att_dst2, b2, nslice=4):
    import ml_dtypes

    bf = ml_dtypes.bfloat16
    nb, ncn, d = x.shape
    N = nb * ncn
    H1, C1 = att_src1.shape
    D1 = H1 * C1
    D2 = W2.shape[1]

    xf = (np.asarray(x, np.float32).reshape(N, d)
          * np.asarray(seq, np.float32).reshape(N, 1))
    src = np.asarray(edges[0], np.int64)
    dst = np.asarray(edges[1], np.int64)
    k, esrc_g, dloc = _edge_schedule(src, dst, N)
    cfg = Cfg(N, d, H1, C1, D2, k, nslice)

    # (c-major) head permutation: new col c*H1+h  <-  old col h*C1+c
    new2old = np.empty(D1, dtype=np.int64)
    for c in range(C1):
        for h in range(H1):
            new2old[c * H1 + h] = h * C1 + c
    w1 = np.asarray(W1, np.float32)
    wsrc = np.einsum("khc,hc->kh", w1.reshape(d, H1, C1), np.asarray(att_src1, np.float32))
    wdst = np.einsum("khc,hc->kh", w1.reshape(d, H1, C1), np.asarray(att_dst1, np.float32))
    wpack1 = np.concatenate([w1[:, new2old], wsrc, wdst], axis=1).astype(bf)

    w2a = np.asarray(W2, np.float32)
    a2s = np.asarray(att_src2, np.float32).reshape(-1)
    a2d = np.asarray(att_dst2, np.float32).reshape(-1)
    c2_const = 0.0  # t2 table is centered on-device; no constant needed
    b2eff = np.asarray(b2, np.float32)

    w2p = w2a[new2old, :]
    w2pack = np.concatenate(
        [w2p, (w2p @ a2s)[:, None], (w2p @ a2d)[:, None]], axis=1).astype(bf)

    b1p = np.tile(np.asarray(b1, np.float32)[new2old][None, :], (128, 1)).astype(np.float32)
    b2effr = np.tile(b2eff[None, :], (128, 1)).astype(np.float32)
    iota = np.tile(np.arange(128, dtype=np.float32)[None, :], (128, 1))
    iotac = np.arange(128, dtype=np.float32)[:, None].copy()
    iota_dk = np.repeat(np.arange(128, dtype=np.float32), k)[None, :]
    iota_dk = np.tile(iota_dk, (128, 1)).astype(bf)

    nblk = cfg.NBLK
    # local dst row per slot: b*128 + dloc (0 for pads)
    dloc_cb = dloc  # [NCORES, nblk, 128, k], -1 for pads
    base = (np.arange(nblk, dtype=np.float32) * BLK)[None, :, None, None]
    edst_local = np.where(dloc_cb >= 0, dloc_cb + base, 0.0).astype(np.int64)

    def to_sb(a, dt):
        # [nblk, 128, k] -> [128, nblk*k]
        return np.ascontiguousarray(
            a.transpose(1, 0, 2).reshape(128, nblk * k)).astype(dt)

    phys = _t2_phys(cfg)
    in_maps = []
    for c in range(NCORES):
        rot = (np.arange(N, dtype=np.int64) + c * cfg.NSH) % N
        xt_c = np.ascontiguousarray(xf[rot].T).astype(bf)
        e1 = ((esrc_g[c] - c * cfg.NSH) % N)
        e2 = phys[esrc_g[c]]
        in_maps.append(
            {
                "xt": xt_c,
                "wpack1": wpack1,
                "w2pack": w2pack,
                "b1p": b1p,
                "b2effr": b2effr,
                "iota": iota,
                "iotac": iotac,
                "iota_dk": iota_dk,
                "esrc1": to_sb(e1, np.int32),
                "esrc2": to_sb(e2, np.int32),
                "edstl": to_sb(edst_local[c], np.int32),
                "edloc": to_sb(dloc_cb[c], bf),
            }
        )
    return cfg, c2_const, in_maps


_CACHE = {}
LAST_RESULT = None


def kernel(**inputs) -> np.ndarray:
    from concourse.bass_utils import run_bass_kernel_spmd

    global LAST_RESULT
    x = np.asarray(inputs["x"])
    nb, ncn, d = x.shape
    nslice = int(os.environ.get("KNSLICE", "4"))
    cfg, c2_const, in_maps = prepare(**{k: inputs[k] for k in (
        "x", "seq", "edges", "W1", "att_src1", "att_dst1", "b1",
        "W2", "att_src2", "att_dst2", "b2")}, nslice=nslice)

    phases = os.environ.get("KPHASES", "abgc")
    key = (cfg.N, cfg.D, cfg.H1, cfg.C1, cfg.D2, cfg.K, cfg.NSLICE,
           round(c2_const, 10), phases)
    if key not in _CACHE:
        _CACHE.clear()
        _CACHE[key] = build_program(cfg, c2_const, phases=phases)
    nc = _CACHE[key]

    res = run_bass_kernel_spmd(nc, in_maps, core_ids=list(range(NCORES)), trace=False)
    LAST_RESULT = res
    shards = [res.results[c]["out"] for c in range(NCORES)]
    full = np.concatenate(shards, axis=0)
    return full.reshape(nb, ncn, d).astype(np.float32)
